# revision 1
# baseline (speedup 1.0000x reference)
"""Trainium2 Bass kernel for nn_MobiusGraphConv (spectral graph conv).

Math: the reference materializes R = eigenVec @ M @ eigenVec^T ([N,N]) and
computes out = 2*Re((R @ input) @ W) + bias.  But M is DIAGONAL complex
(built from elementwise ops on A,B,C,D,eigenVal), so everything factors
through the 16-dim spectral space:

    G  = eigenVec^T @ input                      [16, 32]
    H0 = G @ W0,  H1 = G @ W1                    [16, 32]
    out = 2*((eigenVec*m0) @ H0 - (eigenVec*m1) @ H1) + bias

where m0/m1 are the real/imag diagonals of M (computed on host, O(K)).

Sharding: node dim N=8192 is row-sharded 8 ways for phase 2 (each core
computes its 1024 output rows); the G reduction needs ALL rows, so input
and eigenVec are replicated to every core - cheaper than a cross-core
collective at these sizes.

Device program per core (matmul operands fp16, accumulation fp32 in PSUM;
fp32 matmuls on trn2 lower to two HW passes and forbid fast weight load,
so fp16 operands halve both PE time and DMA bytes; end-to-end rel err
~5.6e-4):
  phase 1: 16 accumulating matmuls computing G^T via 4-chunk blocking
           (lhsT = input chunks [128, 4*32], rhs = eigenVec chunks
           [128, 4*16]); the 4 diagonal [32,16] blocks of the [128,64]
           PSUM are summed on DVE to give G^T in SBUF (fp16).
  algebra: H = G @ [W0|W1] as one matmul (lhsT = G^T); S built [64,32]:
           H0 at partitions 0:16, H1 at 32:48 (compute APs must start at
           partition 0 mod 32), bias row at 48 (ones row in evmt folds
           the bias add into the phase-2 matmul).
  phase 2: 8 matmuls out[128,32] = (evmt chunk).T @ S into two PSUM
           banks; two DVE copies to SBUF (the first overlaps the later
           matmuls - legal because it reads a different psum bank);
           single DMA out in partition-major layout (host un-permutes).

Built as raw bacc with hand-placed semaphores (no Tile): Tile's scheduler
is correct but spends ~8us on its entry/exit barriers and semaphore
resets, which dominates at this kernel's size.  The Bass-init constant
memsets and all-engine barrier are also stripped from the preamble so the
SP engine can issue the input DMAs immediately at kernel entry, hiding
the DMA latency under the other engines' cold instruction fetch.
"""

import os

import numpy as np

import concourse.mybir as mybir
from concourse import bacc, bass_utils

N, K, FIN, FOUT = 8192, 16, 32, 32
NCORES = 8
SHARD = N // NCORES  # 1024 rows per core
NCHUNK = N // 128  # 64 chunks of 128 rows in "(p o)" layout
BLK = 4  # chunks per phase-1 matmul group
NGROUP = NCHUNK // BLK  # 16
EVROWS = 4 * K  # evmt/Scat partition count (padded, see layout above)
OCH = SHARD // 128  # 8 output row-chunks per core

_cache = {}


def _strip_preamble(nc):
    """Remove Bass-init const memsets + the entry all-engine barrier.

    Both are safe to drop here: the consts are never read, and ordering
    is fully carried by this kernel's own semaphores (the runtime only
    starts an execution after the previous one fully quiesced).
    """
    try:
        blk = nc.main_func.blocks[0]
        drop = (mybir.InstMemset, mybir.InstDrain, mybir.InstEventSemaphore)
        keep = [i for i in blk.instructions if not isinstance(i, drop)]
        if 0 < len(blk.instructions) - len(keep) <= 20:
            blk.instructions[:] = keep
    except Exception:
        pass  # stripping is a perf optimization only; never fail the build


def _build_raw():
    f16 = mybir.dt.float16
    f32 = mybir.dt.float32
    nc = bacc.Bacc("TRN2", target_bir_lowering=False, debug=False, num_devices=1)
    _strip_preamble(nc)

    # host-packed phase-1 stream: quarter q holds input chunks 16q..16q+15
    # (512 cols) then eigenVec chunks 16q..16q+15 (256 cols)
    QCOLS = (NCHUNK // 4) * (FIN + K)  # 768
    st_d = nc.dram_tensor("stream", [128, 4 * QCOLS], f16, kind="ExternalInput")
    # merged small tensor: [evmt (1024) | wcat padded to 64 rows (64) |
    # scat template: zeros + bias row (32)]
    SMW = SHARD + 2 * FOUT + FOUT  # 1120
    sm_d = nc.dram_tensor("smalls", [EVROWS, SMW], f16, kind="ExternalInput")
    # partition-major out: out[p, j*32+f] = row (j*128+p) of this shard
    out_d = nc.dram_tensor("out", [128, OCH * FOUT], f32, kind="ExternalOutput")

    St = nc.alloc_sbuf_tensor("St", [128, 4 * QCOLS], f16).ap()
    Sm = nc.alloc_sbuf_tensor("Sm", [EVROWS, SMW], f16).ap()
    EvmT = Sm[:, 0:SHARD]
    Wc = Sm[0:FIN, SHARD : SHARD + 2 * FOUT]
    Scat = Sm[:, SHARD + 2 * FOUT :]
    t0 = nc.alloc_sbuf_tensor("t0", [FIN, K], f32).ap()
    t1 = nc.alloc_sbuf_tensor("t1", [FIN, K], f32).ap()
    t2 = nc.alloc_sbuf_tensor("t2", [FIN, K], f32).ap()
    GT = nc.alloc_sbuf_tensor("GT", [FIN, K], f16).ap()
    Osb = nc.alloc_sbuf_tensor("Osb", [128, OCH * FOUT], f32).ap()

    psum_G = nc.alloc_psum_tensor("psG", [128, BLK * K], f32).ap()
    psum_H = nc.alloc_psum_tensor("psH", [K, 2 * FOUT], f32).ap()
    # phase-2 PSUM in TWO tensors (= two banks): the first PSUM->SBUF copy
    # runs while PE still writes the later chunks, and concurrent PE-write
    # + DVE-read of the SAME psum bank is an electrically fatal conflict -
    # bank-splitting makes the overlap legal
    psum_Oa = nc.alloc_psum_tensor("psOa", [128, OCH * FOUT // 2], f32).ap()
    psum_Ob = nc.alloc_psum_tensor("psOb", [128, OCH * FOUT // 2], f32).ap()

    # NOTE on DMA semaphores: each dma_start's 16 increments come from the
    # 16 SDMA engines independently, and a later DMA's increments on the
    # same ring can land before an earlier DMA's are all in.  A shared
    # counter is therefore only sound at its FULL count, so every DMA
    # below gets its own semaphore waited at 16.
    s_st = nc.alloc_semaphore("s_st")
    s_aux = nc.alloc_semaphore("s_aux")
    s_pe = nc.alloc_semaphore("s_pe")
    s_dve = nc.alloc_semaphore("s_dve")
    s_out = nc.alloc_semaphore("s_out")  # outside the cleared range

    # ONE dma_start for the whole stream (A/B-measured fastest): splitting
    # it - even across both HWDGE rings - only adds per-DMA issue time
    # (~0.6us each) and completion-receipt contention; the transfer time
    # itself is pinned by aggregate HBM bandwidth across the 8 cores, and
    # the ~2us completion receipt is paid once here.
    # smalls go BEHIND the stream on the same SP ring: ring FIFO means
    # their 140KB no longer shares per-core HBM bandwidth with the
    # critical 768KB stream transfer (they are not needed until the
    # H-matmul, ~2us after the stream semaphore - ample slack)
    nc.sync.dma_start(St, st_d.ap()).then_inc(s_st, 16)
    nc.sync.dma_start(Sm, sm_d.ap()).then_inc(s_aux, 16)

    # PE phase 1: G^T accumulation over 16 blocked matmuls
    nc.tensor.wait_ge(s_st, 16)
    for g in range(NGROUP):
        q, j = divmod(g, BLK)
        mm = nc.tensor.matmul(
            psum_G,
            lhsT=St[:, q * QCOLS + j * BLK * FIN : q * QCOLS + (j + 1) * BLK * FIN],
            rhs=St[
                :,
                q * QCOLS + 4 * BLK * FIN + j * BLK * K : q * QCOLS
                + 4 * BLK * FIN
                + (j + 1) * BLK * K,
            ],
            start=(g == 0),
            stop=(g == NGROUP - 1),
        )
    mm.then_inc(s_pe, 1)

    # DVE: sum the 4 diagonal [32,16] blocks -> G^T (fp16 for matmul);
    # chained because DVE may read at most one PSUM operand per op
    nc.vector.wait_ge(s_pe, 1)
    nc.vector.tensor_copy(t0, psum_G[0:32, 0:16])
    nc.vector.tensor_add(t1, psum_G[32:64, 16:32], t0)
    nc.vector.tensor_add(t2, psum_G[64:96, 32:48], t1)
    nc.vector.tensor_add(GT, psum_G[96:128, 48:64], t2).then_inc(s_dve, 1)

    # PE: H = G @ [W0 | W1]
    nc.tensor.wait_ge(s_dve, 1)
    nc.tensor.wait_ge(s_aux, 16)
    nc.tensor.matmul(psum_H, lhsT=GT, rhs=Wc, start=True, stop=True).then_inc(
        s_pe, 1
    )

    # DVE: S = [H0@0:16 ; H1@32:48] over the DMA'd zeros+bias template
    # (any ACT-engine op would pull a ~1.3us ACT_TABLE_LOAD to the head of
    # the ACT stream, delaying its DMA - so both casts stay on DVE)
    nc.vector.wait_ge(s_pe, 2)
    nc.vector.tensor_copy(Scat[0:K, :], psum_H[:, 0:FOUT])
    nc.vector.tensor_copy(Scat[2 * K : 3 * K, :], psum_H[:, FOUT:]).then_inc(
        s_dve, 1
    )

    # PE phase 2: 8 matmuls into two PSUM banks (s_dve>=2 transitively
    # implies s_aux>=16, i.e. EvmT is resident); matmuls complete in
    # order, so a mid-point inc lets the first PSUM->SBUF copy overlap
    # the remaining matmuls
    nc.tensor.wait_ge(s_dve, 2)
    for j in range(OCH):
        ps = psum_Oa if j < OCH // 2 else psum_Ob
        jj = j % (OCH // 2)
        mm = nc.tensor.matmul(
            ps[:, jj * FOUT : (jj + 1) * FOUT],
            lhsT=EvmT[:, j * 128 : (j + 1) * 128],
            rhs=Scat,
            start=True,
            stop=True,
        )
        if j == OCH // 2 - 1:
            mm.then_inc(s_pe, 1)
    mm.then_inc(s_pe, 1)

    # DVE: PSUM -> SBUF in two halves (DMA cannot read PSUM)
    HALF = OCH * FOUT // 2
    nc.vector.wait_ge(s_pe, 3)
    nc.vector.tensor_copy(Osb[:, 0:HALF], psum_Oa)
    nc.vector.wait_ge(s_pe, 4)
    nc.vector.tensor_copy(Osb[:, HALF:], psum_Ob).then_inc(s_dve, 1)

    # SP: reset semaphores (all their increments have landed: every wait
    # above was a full-count wait), then write out.  The runtime's exit
    # drain covers the out-DMA's completion, so nothing waits on it;
    # s_out is never waited or cleared - its residue is unused state.
    nc.sync.wait_ge(s_dve, 3)
    nc.sync.sem_clear(range(s_st.num, s_dve.num + 1))
    nc.sync.dma_start(out_d.ap(), Osb).then_inc(s_out, 16)

    nc.compile()
    return nc


def _host_prep(input, eigenVal, eigenVec, A, B, C, D, W, bias):
    """Host spectral core: M is diagonal complex; fold into eigenVec shards."""
    ev = eigenVal.astype(np.float64)
    m1r = A[0] * ev + B[0]
    m1i = A[1] * ev + B[1]
    invr = 1.0 / (C[0] * ev + D[0])
    invi = 1.0 / (C[1] * ev + D[1])
    m0d = (m1r * invr - m1i * invi).astype(np.float32)
    m1d = (m1i * invr + m1r * invi).astype(np.float32)

    # phase-1 stream, packed per quarter: [in chunks 16q..16q+15 | ev ...]
    inp_po = input.astype(np.float16).reshape(128, NCHUNK, FIN)
    ev_po = eigenVec.astype(np.float16).reshape(128, NCHUNK, K)
    pieces = []
    for q in range(4):
        pieces.append(inp_po[:, 16 * q : 16 * (q + 1)].reshape(128, 16 * FIN))
        pieces.append(ev_po[:, 16 * q : 16 * (q + 1)].reshape(128, 16 * K))
    stream = np.ascontiguousarray(np.concatenate(pieces, 1))  # [128, 3072]

    wcat = np.concatenate([W[0], W[1]], 1).astype(np.float16)  # [32, 64]
    smalls = []
    for c in range(NCORES):
        sl = eigenVec[c * SHARD : (c + 1) * SHARD]  # [1024, 16]
        sm = np.zeros((EVROWS, SHARD + 3 * FOUT), np.float16)
        sm[0:K, 0:SHARD] = (2.0 * sl * m0d).T
        sm[2 * K : 3 * K, 0:SHARD] = (-2.0 * sl * m1d).T
        sm[3 * K, 0:SHARD] = 1.0  # ones row: folds bias into phase 2
        sm[0:FIN, SHARD : SHARD + 2 * FOUT] = wcat
        sm[3 * K, SHARD + 2 * FOUT :] = bias.astype(np.float16)
        smalls.append(sm)
    return stream, smalls


last_results = None  # BassKernelResults of the most recent run (for test.py)


def kernel(input, eigenVal, eigenVec, W, A, B, C, D, bias):
    global last_results
    input = np.ascontiguousarray(np.asarray(input), np.float32)
    eigenVal = np.asarray(eigenVal, np.float32)
    eigenVec = np.ascontiguousarray(np.asarray(eigenVec), np.float32)
    W = np.asarray(W, np.float32)
    A = np.asarray(A, np.float32)
    B = np.asarray(B, np.float32)
    C = np.asarray(C, np.float32)
    D = np.asarray(D, np.float32)
    bias = np.asarray(bias, np.float32)

    if "nc" not in _cache:
        _cache["nc"] = _build_raw()
    nc = _cache["nc"]

    stream, smalls = _host_prep(
        input, eigenVal, eigenVec, A, B, C, D, W, bias
    )
    in_maps = [{"stream": stream, "smalls": smalls[c]} for c in range(NCORES)]

    trace = os.environ.get("KERNEL_TRACE", "0") == "1"
    if trace:
        _install_ntff_hook()

    res = bass_utils.run_bass_kernel_spmd(
        nc,
        in_maps,
        core_ids=list(range(NCORES)),
        trace=trace,
        trace_cores=list(range(NCORES)) if trace else None,
    )
    last_results = res

    # un-permute: out[p, j*32+f] = row (j*128+p) -> [1024, 32] per core
    shards = []
    for c in range(NCORES):
        o = res.results[c]["out"].reshape(128, OCH, FOUT)
        shards.append(o.transpose(1, 0, 2).reshape(SHARD, FOUT))
    return np.concatenate(shards, 0).reshape(1, N, FOUT)


def _install_ntff_hook():
    """The image's antenv lacks axon_hooks; register the NTFF profile hook
    (needed only for trace=True) by injecting the shim module."""
    import sys
    import types

    if "antenv.axon_hooks" in sys.modules:
        return
    holder = {"h": None}
    mod = types.ModuleType("antenv.axon_hooks")
    mod.set_axon_ntff_profile_hook = lambda h: holder.__setitem__("h", h)
    mod.get_axon_ntff_profile_hook = lambda: holder["h"]
    sys.modules["antenv.axon_hooks"] = mod
    import antenv

    antenv.axon_hooks = mod
    try:
        from trn_agent_boot.trn_boot import _ntff_profile_via_ctypes

        mod.set_axon_ntff_profile_hook(
            _ntff_profile_via_ctypes("/opt/axon/libaxon_pjrt.so")
        )
    except Exception:
        pass

